# revision 45
# baseline (speedup 1.0000x reference)
"""GAT (3-layer, PyG-style) Trainium2 Bass kernel, sharded across 8 NeuronCores.

Destination-node range partition (graph parallel), one SPMD program.
Per layer each core computes h_ext = X_own @ [W | W.a_s | W.a_d] for its
nodes (bf16 matmuls; the next layer's h_ext is fused into the edge loop so
the AllGather of the halo table overlaps edge aggregation). Tables: L0/L2
bf16; L1 rows are 768B mixed-precision (512 fp8 h | 16B bf16 att) - fp8 on
L0 was too lossy. A separate bf16 hloc strip keeps self-loop rows full
precision. Edge aggregation per pair of dst tiles: one lo + one hi
supergather (dma_gather, 0-padded idx; single_packet=False required above
~64 descs/engine), host-prebuilt fp8 one-hot S/St matrices, al_d per edge
via St matmul from a persistent SBUF al_d strip, softmax weights
max(exp(z), exp(0.2 z)) on ACT, weighted rows on DVE (per-head broadcast),
and the scatter runs as S matmuls accumulating numerator + denominator in
PSUM, with the self-loop as one extra identity-matrix block. AllGathers are
Pool-triggered (HW requirement, collectives are Pool-engine-only); each
layer's hi-region trigger fires at its edge loop's head, where its CC-stream
wait costs nothing because the lead gathers block on the lo AG anyway.

kernel(**inputs) takes the FULL inputs and returns the FULL [N, 16] output.
"""

import sys

sys.path.insert(0, "/opt/trn_rl_repo")

import numpy as np

import concourse.bass as bass
import concourse.mybir as mybir
import concourse.tile as tile
from concourse import bacc
from concourse import bass_utils
from concourse.bass_interp import get_hw_module
from concourse.masks import make_identity
from concourse import library_config

F32 = mybir.dt.float32
BF = mybir.dt.bfloat16
FP8 = mybir.dt.float8e4
I16 = mybir.dt.int16
import ml_dtypes
NPBF = ml_dtypes.bfloat16
NPF8 = ml_dtypes.float8_e4m3
P = 128
ALU = mybir.AluOpType
ACTF = mybir.ActivationFunctionType


def real_cfg():
    R = 8
    N = 50000
    PER = N // R                      # 6250 nodes per core
    T = (PER + P - 1) // P            # 49 dst tiles per core
    return dict(
        R=R, N=N, PER=PER, T=T, NPAD=T * P,
        F_IN=128, HID=64, HEADS=8, N_CLASSES=16,
        NEG=0.2,
        SPLIT_T=25,                   # lo group = tiles 0..24
        K_LEAD=6,                     # lo-gathers issued K tiles ahead
    )


# ---------------------------------------------------------------------------
# Host-side preprocessing
# ---------------------------------------------------------------------------

def _wrap16(flat):
    """int16 index list -> dma_gather idx layout [128, n/16]."""
    n = flat.shape[-1]
    w = flat.reshape(flat.shape[:-1] + (n // 16, 16))
    w = np.swapaxes(w, -1, -2)
    reps = (1,) * (flat.ndim - 1) + (8, 1)
    return np.ascontiguousarray(np.tile(w, reps), np.int16)


def host_prepare(inputs, cfg):
    R, N, PER, T, NPAD = cfg["R"], cfg["N"], cfg["PER"], cfg["T"], cfg["NPAD"]
    F_IN, HID, HEADS, NCLS = cfg["F_IN"], cfg["HID"], cfg["HEADS"], cfg["N_CLASSES"]
    HC = HID * HEADS                  # 512
    SPLIT_T = cfg["SPLIT_T"]
    LO = SPLIT_T * P                  # 3968
    HI = NPAD - LO                    # 2304

    x = np.asarray(inputs["x"], np.float32)
    ei = np.asarray(inputs["edge_index"])
    src = ei[0].astype(np.int64)
    dst = ei[1].astype(np.int64)      # self-loops handled analytically on device

    core = dst // PER
    dloc = (dst - core * PER).astype(np.int64)
    sloc = (src % PER).astype(np.int64)
    srank = (src // PER).astype(np.int64)
    is_lo = sloc < LO
    tile_of = dloc // P

    # gather row id within the lo / hi tables
    grow = np.where(is_lo, srank * LO + sloc, srank * HI + (sloc - LO))

    # per (core, tile, group) counts -> per-tile block counts (max over cores)
    cl = np.zeros((R, T), np.int64)
    ch = np.zeros((R, T), np.int64)
    np.add.at(cl, (core[is_lo], tile_of[is_lo]), 1)
    np.add.at(ch, (core[~is_lo], tile_of[~is_lo]), 1)
    BLt = np.maximum(1, np.ceil(cl.max(axis=0) / P).astype(np.int64))  # [T]
    BHt = np.maximum(1, np.ceil(ch.max(axis=0) / P).astype(np.int64))
    Bt = BLt + BHt
    BLM, BHM, BTM = int(BLt.max()), int(BHt.max()), int(Bt.max())

    # flat layout offsets (shared by all cores; shapes must be SPMD-static)
    off_lo = np.concatenate([[0], np.cumsum(BLt * 8)]).astype(np.int64)
    off_hi = np.concatenate([[0], np.cumsum(BHt * 8)]).astype(np.int64)
    off_S = np.concatenate([[0], np.cumsum(Bt * P)]).astype(np.int64)
    off_m8 = np.concatenate([[0], np.cumsum(Bt * 8)]).astype(np.int64)
    off_m1 = np.concatenate([[0], np.cumsum(Bt)]).astype(np.int64)

    # order edges by (core, tile, group), positions within each group
    order = np.lexsort((~is_lo * 1, tile_of, core))
    g_s = grow[order]
    d_s = dloc[order]
    core_s = core[order]
    tile_s = tile_of[order]
    lo_s = is_lo[order]
    grp = core_s * (2 * T) + tile_s * 2 + (~lo_s).astype(np.int64)
    grp_start = np.searchsorted(grp, np.arange(R * T * 2), side="left")
    pos = np.arange(len(grp)) - grp_start[grp]

    # NOTE: pad slots use index 0 (gather real row 0). Trailing -1 indices
    # would be truncated by the ucode, but that corrupts SWDGE ring
    # accounting when many gathers are in flight (HW hang).
    ilo = np.zeros((R, int((BLt * P).sum())), np.int16)
    ihi = np.zeros((R, int((BHt * P).sum())), np.int16)
    slot_lo = np.concatenate([[0], np.cumsum(BLt * P)]).astype(np.int64)
    slot_hi = np.concatenate([[0], np.cumsum(BHt * P)]).astype(np.int64)

    lo_m = lo_s
    hi_m = ~lo_s
    ilo[core_s[lo_m], slot_lo[tile_s[lo_m]] + pos[lo_m]] = g_s[lo_m].astype(np.int16)
    ihi[core_s[hi_m], slot_hi[tile_s[hi_m]] + pos[hi_m]] = g_s[hi_m].astype(np.int16)

    # S (scatter, [slot, 128 dst]) / St (transposed) / masks, flat by tile.
    # slot index within tile t: lo slots [0, BLt*P), hi slots [BLt*P, Bt*P)
    nS = int((Bt * P).sum())
    S_np = np.zeros((R, P, nS), NPF8)
    St_np = np.zeros((R, P, nS), NPF8)
    fpos = np.where(lo_m, pos, BLt[tile_s] * P + pos)  # slot within tile
    blk = fpos // P
    lane = fpos % P
    S_cols = off_S[tile_s] + blk * P + (d_s - tile_s * P)
    S_np[core_s, lane, S_cols] = 1.0
    St_np[core_s, (d_s - tile_s * P), off_S[tile_s] + blk * P + lane] = 1.0

    # wrap idx per tile -> [128, BLt*8] blocks, concat -> [128, sum]
    ilo_w = np.concatenate(
        [_wrap16(ilo[:, slot_lo[t]:slot_lo[t + 1]]) for t in range(T)], axis=-1)
    ihi_w = np.concatenate(
        [_wrap16(ihi[:, slot_hi[t]:slot_hi[t + 1]]) for t in range(T)], axis=-1)

    # weight assembly: W'[f, :] = [W | W.a_src | W.a_dst]
    def wext(W, a_s, a_d, ncols):
        Fin = W.shape[0]
        H, C = a_s.shape
        Wr = W.reshape(Fin, H, C)
        We = np.zeros((Fin, ncols), np.float32)
        We[:, : H * C] = W
        We[:, H * C: H * C + H] = np.einsum("fhc,hc->fh", Wr, a_s)
        We[:, H * C + H: H * C + 2 * H] = np.einsum("fhc,hc->fh", Wr, a_d)
        return We

    W0e = wext(np.asarray(inputs["W0"], np.float32),
               np.asarray(inputs["a_s0"], np.float32),
               np.asarray(inputs["a_d0"], np.float32), 528).astype(NPBF)
    W1e = wext(np.asarray(inputs["W1"], np.float32),
               np.asarray(inputs["a_s1"], np.float32),
               np.asarray(inputs["a_d1"], np.float32), 528)
    W2e = wext(np.asarray(inputs["W2"], np.float32),
               np.asarray(inputs["a_s2"], np.float32),
               np.asarray(inputs["a_d2"], np.float32), 18)
    W1e_r = W1e.reshape(4, P, 528).transpose(1, 0, 2).astype(NPBF).copy()
    W2p = np.zeros((512, 32), np.float32)
    W2p[:, :18] = W2e
    W2e_r = W2p.reshape(4, P, 32).transpose(1, 0, 2).astype(NPBF).copy()

    def bext(b, ncols):
        be = np.zeros((1, ncols), np.float32)
        be[0, : b.shape[0]] = b
        return np.ascontiguousarray(np.broadcast_to(be, (P, ncols)))

    b0e = bext(np.asarray(inputs["b0"], np.float32), 528)
    b1e = bext(np.asarray(inputs["b1"], np.float32), 528)
    b2e = bext(np.asarray(inputs["b2"], np.float32), 32)

    in_maps = []
    for r in range(R):
        xt = np.zeros((F_IN, NPAD), np.float32)
        xt[:, :PER] = x[r * PER: (r + 1) * PER].T
        in_maps.append({
            "xt0": xt.astype(NPBF),
            "w0e": W0e, "w1e": W1e_r, "w2e": W2e_r,
            "b0e": b0e, "b1e": b1e, "b2e": b2e,
            "ilo": ilo_w[r], "ihi": ihi_w[r],
            "S": S_np[r], "St": St_np[r],
        })
    meta = dict(BLt=BLt.tolist(), BHt=BHt.tolist(), Bt=Bt.tolist(),
                BLM=BLM, BHM=BHM, BTM=BTM,
                off_lo=off_lo.tolist(), off_hi=off_hi.tolist(),
                off_S=off_S.tolist(), off_m8=off_m8.tolist(),
                off_m1=off_m1.tolist())
    return in_maps, meta


# ---------------------------------------------------------------------------
# Device program
# ---------------------------------------------------------------------------

def build_gat_nc(cfg, meta, stage=4):
    R, PER, T, NPAD = cfg["R"], cfg["PER"], cfg["T"], cfg["NPAD"]
    F_IN, HID, HEADS, NCLS = cfg["F_IN"], cfg["HID"], cfg["HEADS"], cfg["N_CLASSES"]
    NEG = cfg["NEG"]
    HC = HID * HEADS
    SPLIT_T = cfg["SPLIT_T"]
    LO = SPLIT_T * P
    HI = NPAD - LO
    K = cfg["K_LEAD"]
    BLt, BHt, Bt = meta["BLt"], meta["BHt"], meta["Bt"]
    BLM, BHM, BTM = meta["BLM"], meta["BHM"], meta["BTM"]
    off_lo, off_hi = meta["off_lo"], meta["off_hi"]
    off_S, off_m8, off_m1 = meta["off_S"], meta["off_m8"], meta["off_m1"]

    nc = bacc.Bacc("TRN2", target_bir_lowering=False, debug=False,
                   num_devices=R)

    xt0_d = nc.dram_tensor("xt0", [F_IN, NPAD], BF, kind="ExternalInput")
    w0e_d = nc.dram_tensor("w0e", [P, 528], BF, kind="ExternalInput")
    w1e_d = nc.dram_tensor("w1e", [P, 4, 528], BF, kind="ExternalInput")
    w2e_d = nc.dram_tensor("w2e", [P, 4, 32], BF, kind="ExternalInput")
    b0e_d = nc.dram_tensor("b0e", [P, 528], F32, kind="ExternalInput")
    b1e_d = nc.dram_tensor("b1e", [P, 528], F32, kind="ExternalInput")
    b2e_d = nc.dram_tensor("b2e", [P, 32], F32, kind="ExternalInput")
    ilo_d = nc.dram_tensor("ilo", [P, off_lo[-1]], I16, kind="ExternalInput")
    ihi_d = nc.dram_tensor("ihi", [P, off_hi[-1]], I16, kind="ExternalInput")
    S_d = nc.dram_tensor("S", [P, off_S[-1]], FP8, kind="ExternalInput")
    St_d = nc.dram_tensor("St", [P, off_S[-1]], FP8, kind="ExternalInput")
    out_d = nc.dram_tensor("out", [PER, NCLS], F32, kind="ExternalOutput")

    rg = [list(range(R))]
    ROWW = [640, 768, 128]            # table row width per layer (in elems)
    TDT = [BF, FP8, BF]               # table dtype per layer
    NC_L = [HC, HC, NCLS]             # value cols per layer
    NH_L = [HEADS, HEADS, 1]

    with tile.TileContext(nc) as tc:
        with (
            tc.tile_pool(name="pers", bufs=1) as pers,
            tc.tile_pool(name="ld", bufs=3) as ld,        # S/St/mask/loc loads
            tc.tile_pool(name="ldi", bufs=7) as ldi,  # idx loads
            tc.tile_pool(name="glo", bufs=4) as gloP,
            tc.tile_pool(name="ghi", bufs=3) as ghiP,
            tc.tile_pool(name="wk", bufs=2) as wk,        # small working tiles
            tc.tile_pool(name="gw", bufs=2) as gwP,
            tc.tile_pool(name="hx", bufs=2) as hx,        # hsb / xe tiles
            tc.tile_pool(name="po", bufs=2, space="PSUM") as poP,
            tc.tile_pool(name="ps_sm", bufs=2, space="PSUM") as psS,
            tc.tile_pool(name="ph", bufs=2, space="PSUM") as phP,
            tc.tile_pool(name="phb", bufs=1, space="PSUM") as phbP,
            tc.tile_pool(name="pt", bufs=1, space="PSUM") as ptP,
            tc.tile_pool(name="dram", bufs=1, space="DRAM") as dram,
        ):
            nc.gpsimd.load_library(library_config.mlp)

            # ---- persistent tiles ----
            ident = pers.tile([P, P], BF)
            make_identity(nc, ident[:])

            w0_sb = pers.tile([P, 528], BF)
            w1_sb = pers.tile([P, 4, 528], BF)
            w2_sb = pers.tile([P, 4, 32], BF)
            b0_sb = pers.tile([P, 528], F32)
            b1_sb = pers.tile([P, 528], F32)
            b2_sb = pers.tile([P, 32], F32)
            nc.sync.dma_start(w0_sb[:], w0e_d[:, :])
            nc.sync.dma_start(w1_sb[:], w1e_d[:, :, :])
            nc.sync.dma_start(w2_sb[:], w2e_d[:, :, :])
            nc.sync.dma_start(b0_sb[:], b0e_d[:, :])
            nc.sync.dma_start(b1_sb[:], b1e_d[:, :])
            nc.sync.dma_start(b2_sb[:], b2e_d[:, :])

            Xt0 = pers.tile([P, NPAD], BF)
            nc.sync.dma_start(Xt0[:], xt0_d[:, :])

            # al_d strips: [128, T*nH] per layer
            ald = [pers.tile([P, T * 8], BF, name=f"ald{L}") for L in range(3)]
            for L in range(3):
                nc.vector.memset(ald[L][:], 0.0)

            # ---- internal DRAM ----
            hA = [dram.tile([LO, ROWW[L]], TDT[L], name=f"hA{L}") for L in range(3)]
            hB = [dram.tile([HI, ROWW[L]], TDT[L], name=f"hB{L}") for L in range(3)]
            tA = [dram.tile([R * LO, ROWW[L]], TDT[L], addr_space="Shared",
                            name=f"tA{L}") for L in range(3)]
            tB = [dram.tile([R * HI, ROWW[L]], TDT[L], addr_space="Shared",
                            name=f"tB{L}") for L in range(3)]
            hloc = {1: dram.tile([NPAD, 528], BF, name="hloc1")}


            def fire_ag(L, region):
                ins_ap, outs_ap = ((hA[L][:, :], tA[L][:, :]) if region == 0
                                   else (hB[L][:, :], tB[L][:, :]))
                nc.gpsimd.collective_compute(
                    "AllGather", ALU.bypass, replica_groups=rg,
                    ins=[ins_ap], outs=[outs_ap])

            def h_tile(L, nt, src_sb):
                """h_ext for layer L, dst tile nt, from feature-major src_sb
                ([P, kc, P] kc chunks of lhsT). Writes table row block + ald."""
                roww = ROWW[L]
                ncol = NC_L[L]
                nh = NH_L[L]
                tot = ncol + 2 * nh
                W_sb = [w0_sb, w1_sb, w2_sb][L]
                b_sb = [b0_sb, b1_sb, b2_sb][L]
                KC = 1 if L == 0 else 4
                pha = phP.tile([P, 512], F32, tag="pha")
                phb = phbP.tile([P, 64], F32, tag="phb")
                n1 = min(512, tot)
                for kc in range(KC):
                    lhs = (Xt0[:, nt * P:(nt + 1) * P] if L == 0
                           else src_sb[:, kc, :])
                    rhs = (w0_sb[:, 0:n1] if L == 0
                           else W_sb[:, kc, 0:n1])
                    nc.tensor.matmul(pha[:, 0:n1], lhsT=lhs, rhs=rhs,
                                     start=(kc == 0), stop=(kc == KC - 1))
                if tot > 512:
                    for kc in range(KC):
                        lhs = (Xt0[:, nt * P:(nt + 1) * P] if L == 0
                               else src_sb[:, kc, :])
                        rhs = (w0_sb[:, 512:tot] if L == 0
                               else W_sb[:, kc, 512:tot])
                        nc.tensor.matmul(phb[:, 0:tot - 512], lhsT=lhs, rhs=rhs,
                                         start=(kc == 0), stop=(kc == KC - 1))
                hsb = hx.tile([P, 528], BF, tag="hsb")
                if tot > 512:
                    nc.vector.tensor_tensor(hsb[:, 0:512], pha[:, 0:512],
                                            b_sb[:, 0:512], ALU.add)
                    nc.vector.tensor_tensor(hsb[:, 512:tot], phb[:, 0:tot - 512],
                                            b_sb[:, 512:tot], ALU.add)
                else:
                    nc.vector.tensor_tensor(hsb[:, 0:tot], pha[:, 0:tot],
                                            b_sb[:, 0:tot], ALU.add)
                # al_d strip
                nc.scalar.copy(ald[L][:, nt * 8:nt * 8 + nh],
                               hsb[:, ncol + nh:tot])
                # table row block
                if TDT[L] == FP8:
                    f8 = hx.tile([P, 512], FP8, tag="hf8")
                    nc.scalar.activation(f8[:], hsb[:, 0:512], ACTF.Copy)
                    att8 = hsb[:, 512:528].bitcast(FP8)
                    dstT = hA[L] if nt < SPLIT_T else hB[L]
                    r0 = nt * P if nt < SPLIT_T else nt * P - LO
                    nc.sync.dma_start(dstT[r0:r0 + P, 0:512], f8[:])
                    nc.sync.dma_start(dstT[r0:r0 + P, 512:544], att8)
                    nc.sync.dma_start(hloc[L][nt * P:(nt + 1) * P, :],
                                      hsb[:, 0:528])
                elif nt < SPLIT_T:
                    nc.sync.dma_start(hA[L][nt * P:(nt + 1) * P, 0:tot],
                                      hsb[:, 0:tot])
                else:
                    r0 = nt * P - LO
                    nc.sync.dma_start(hB[L][r0:r0 + P, 0:tot], hsb[:, 0:tot])
                if nt == SPLIT_T - 1:
                    fire_ag(L, 0)

            # =========== L0 h phase (standalone) ===========
            for nt in range(T):
                h_tile(0, nt, None)

            # =========== edge phases (fused with next layer h) ===========
            def emit_glo(L, s):
                # supergather: one gather covering tiles 2s and 2s+1
                roww = ROWW[L]
                t0 = 2 * s
                t1 = min(2 * s + 2, T)
                bl = sum(BLt[t0:t1])
                it = ldi.tile([P, 2 * BLM * 8], I16, tag="ilo")
                nc.sync.dma_start(it[:, 0:bl * 8], ilo_d[:, off_lo[t0]:off_lo[t1]])
                g = gloP.tile([P, 2 * BLM, roww], TDT[L], tag="glo")
                nc.gpsimd.dma_gather(
                    g[:, 0:bl, :], tA[L][:, :], it[:, 0:bl * 8],
                    num_idxs=bl * P, num_idxs_reg=bl * P, elem_size=roww,
                    single_packet=False)
                return g

            def emit_ghi(L, s):
                roww = ROWW[L]
                t0 = 2 * s
                t1 = min(2 * s + 2, T)
                bh = sum(BHt[t0:t1])
                it = ldi.tile([P, 2 * BHM * 8], I16, tag="ihi")
                nc.sync.dma_start(it[:, 0:bh * 8], ihi_d[:, off_hi[t0]:off_hi[t1]])
                gh = ghiP.tile([P, 2 * BHM, roww], TDT[L], tag="ghi")
                nc.gpsimd.dma_gather(
                    gh[:, 0:bh, :], tB[L][:, :], it[:, 0:bh * 8],
                    num_idxs=bh * P, num_idxs_reg=bh * P, elem_size=roww,
                    single_packet=False)
                return gh

            def edge_tile(L, t, g, gh):
                roww = ROWW[L]
                ncol = NC_L[L]
                nh = NH_L[L]
                tot = ncol + 2 * nh
                bl, bh, bt = BLt[t], BHt[t], Bt[t]
                alow = ncol            # al_s col offset in table rows

                S_sb = ld.tile([P, BTM * P], FP8, tag="S")
                St_sb = ld.tile([P, BTM * P], FP8, tag="St")
                nc.sync.dma_start(S_sb[:, 0:bt * P], S_d[:, off_S[t]:off_S[t + 1]])
                nc.sync.dma_start(St_sb[:, 0:bt * P], St_d[:, off_S[t]:off_S[t + 1]])
                loc = ld.tile([P, 528], BF, tag="loc")
                if TDT[L] == FP8:
                    nc.sync.dma_start(loc[:, 0:tot],
                                      hloc[L][t * P:(t + 1) * P, 0:tot])
                elif t < SPLIT_T:
                    nc.sync.dma_start(loc[:, 0:tot], hA[L][t * P:(t + 1) * P, 0:tot])
                else:
                    r0 = t * P - LO
                    nc.sync.dma_start(loc[:, 0:tot], hB[L][r0:r0 + P, 0:tot])

                # ---- al_d per edge: psmall[:, b*nh:(b+1)*nh] = St_b.T @ ald ----
                psm = psS.tile([P, (BTM + 1) * 8], F32, tag="psm")
                for b in range(bt):
                    nc.tensor.matmul(
                        psm[:, b * nh:(b + 1) * nh],
                        lhsT=St_sb[:, b * P:(b + 1) * P],
                        rhs=ald[L][:, t * 8:t * 8 + nh],
                        start=True, stop=True)

                # ---- logits, weights ----
                z = wk.tile([P, BTM * 8], F32, tag="z")
                if TDT[L] == FP8:
                    galsv_lo = g[:, 0:bl, :].bitcast(BF)[:, :, 256:256 + nh]
                    galsv_hi = gh[:, 0:bh, :].bitcast(BF)[:, :, 256:256 + nh]
                else:
                    galsv_lo = g[:, 0:bl, alow:alow + nh]
                    galsv_hi = gh[:, 0:bh, alow:alow + nh]
                nc.vector.tensor_tensor(
                    z[:, 0:bl * nh].rearrange("p (b h) -> p b h", b=bl),
                    galsv_lo, psm[:, 0:bl * nh].rearrange("p (b h) -> p b h", b=bl),
                    ALU.add)
                nc.vector.tensor_tensor(
                    z[:, bl * nh:bt * nh].rearrange("p (b h) -> p b h", b=bh),
                    galsv_hi,
                    psm[:, bl * nh:bt * nh].rearrange("p (b h) -> p b h", b=bh),
                    ALU.add)
                e1 = wk.tile([P, BTM * 8], F32, tag="e1")
                e2 = wk.tile([P, BTM * 8], F32, tag="e2")
                nc.scalar.activation(e1[:, 0:bt * nh], z[:, 0:bt * nh], ACTF.Exp)
                nc.scalar.activation(e2[:, 0:bt * nh], z[:, 0:bt * nh], ACTF.Exp,
                                     scale=NEG)
                gw = gwP.tile([P, BTM + 1, 520], BF, tag="gw")
                # weights -> gw[:, b, 512:512+nh] (pad slots killed by S zeros)
                nc.vector.tensor_tensor(
                    gw[:, 0:bt, 512:512 + nh],
                    e1[:, 0:bt * nh].rearrange("p (b h) -> p b h", b=bt),
                    e2[:, 0:bt * nh].rearrange("p (b h) -> p b h", b=bt),
                    ALU.max)

                # ---- self-loop weight ----
                sl = wk.tile([P, 3 * 8], F32, tag="sl")
                nc.vector.tensor_tensor(sl[:, 0:nh], loc[:, alow:alow + nh],
                                        ald[L][:, t * 8:t * 8 + nh], ALU.add)
                nc.scalar.activation(sl[:, 8:8 + nh], sl[:, 0:nh], ACTF.Exp)
                nc.scalar.activation(sl[:, 16:16 + nh], sl[:, 0:nh], ACTF.Exp,
                                     scale=NEG)
                nc.vector.tensor_tensor(gw[:, bt, 512:512 + nh],
                                        sl[:, 8:8 + nh], sl[:, 16:16 + nh],
                                        ALU.max)

                # ---- weighted rows ----
                nc.vector.tensor_tensor(
                    gw[:, 0:bl, 0:ncol].rearrange("p b (h c) -> p b h c", h=nh),
                    g[:, 0:bl, 0:ncol].rearrange("p b (h c) -> p b h c", h=nh),
                    gw[:, 0:bl, 512:512 + nh]
                        .unsqueeze(3).to_broadcast([P, bl, nh, ncol // nh]),
                    ALU.mult)
                nc.vector.tensor_tensor(
                    gw[:, bl:bt, 0:ncol].rearrange("p b (h c) -> p b h c", h=nh),
                    gh[:, 0:bh, 0:ncol].rearrange("p b (h c) -> p b h c", h=nh),
                    gw[:, bl:bt, 512:512 + nh]
                        .unsqueeze(3).to_broadcast([P, bh, nh, ncol // nh]),
                    ALU.mult)
                nc.vector.tensor_tensor(
                    gw[:, bt, 0:ncol].rearrange("p (h c) -> p h c", h=nh),
                    loc[:, 0:ncol].rearrange("p (h c) -> p h c", h=nh),
                    gw[:, bt, 512:512 + nh]
                        .unsqueeze(2).to_broadcast([P, nh, ncol // nh]),
                    ALU.mult)

                # ---- scatter matmuls: numerator + denominator ----
                po = poP.tile([P, 512], F32, tag="po")
                for b in range(bt + 1):
                    lhs = (S_sb[:, b * P:(b + 1) * P] if b < bt else ident[:])
                    nc.tensor.matmul(po[:, 0:ncol], lhsT=lhs,
                                     rhs=gw[:, b, 0:ncol],
                                     start=(b == 0), stop=(b == bt))
                    nc.tensor.matmul(psm[:, BTM * 8:BTM * 8 + nh], lhsT=lhs,
                                     rhs=gw[:, b, 512:512 + nh],
                                     start=(b == 0), stop=(b == bt))

                rden = wk.tile([P, 8], F32, tag="rden")
                nc.vector.reciprocal(rden[:, 0:nh], psm[:, BTM * 8:BTM * 8 + nh])
                xn = hx.tile([P, 512], F32, tag="xn")
                nc.vector.tensor_tensor(
                    xn[:, 0:ncol].rearrange("p (h c) -> p h c", h=nh),
                    po[:, 0:ncol].rearrange("p (h c) -> p h c", h=nh),
                    rden[:, 0:nh].unsqueeze(2).to_broadcast([P, nh, ncol // nh]),
                    ALU.mult)
                return xn

            if stage == 31:
                # debug: dump tA[0] rows 0:128 (fp8) as f32 into out
                dbg = pers.tile([P, 768], FP8, name="dbg8")
                dbgf = pers.tile([P, 768], F32, name="dbgf")
                nc.sync.dma_start(dbg[:], tA[0][0:P, :])
                nc.scalar.activation(dbgf[:], dbg[:], ACTF.Copy)
                for j in range(47):
                    nc.sync.dma_start(out_d[j * P:(j + 1) * P, :],
                                      dbgf[:, j * 16:(j + 1) * 16])
                dbga = pers.tile([P, 16], F32, name="dbga")
                nc.scalar.activation(dbga[:], dbg[:].bitcast(BF)[:, 256:272],
                                     ACTF.Copy)
                nc.sync.dma_start(out_d[47 * P:48 * P, :], dbga[:])

            # ---- L0 / L1 edge loops (fused with next h) ----
            NS = (T + 1) // 2
            KS = 3
            for L in ((0, 1) if (stage >= 3 and stage < 30) else ((0,) if 20 <= stage < 30 else ())):
                # hi-region AG for THIS layer's table: the lead gathers below
                # block on the lo-region AG anyway, so firing first costs
                # nothing and starts the hi AG as soon as the CC stream frees
                fire_ag(L, 1)
                pend = []
                for sp in range(min(KS, NS)):
                    pend.append(emit_glo(L, sp))
                for sp in range(NS):
                    g = pend.pop(0)
                    if sp + KS < NS:
                        pend.append(emit_glo(L, sp + KS))
                    gh = emit_ghi(L, sp)
                    for t in range(2 * sp, min(2 * sp + 2, T)):
                        ol = 0 if t == 2 * sp else BLt[2 * sp]
                        oh = 0 if t == 2 * sp else BHt[2 * sp]
                        gv = g[:, ol:ol + BLt[t], :]
                        ghv = gh[:, oh:oh + BHt[t], :]
                        xn = edge_tile(L, t, gv, ghv)
                        # ELU -> xe (bf16)
                        m = wk.tile([P, 512], F32, tag="elu_m")
                        nc.scalar.activation(m[:], xn[:], ACTF.Relu, scale=-1.0)
                        em = wk.tile([P, 512], F32, tag="elu_e")
                        nc.scalar.activation(em[:], m[:], ACTF.Exp, scale=-1.0)
                        rp = wk.tile([P, 512], F32, tag="elu_r")
                        nc.scalar.activation(rp[:], xn[:], ACTF.Relu)
                        xe = hx.tile([P, 512], BF, tag="xe")
                        nc.vector.scalar_tensor_tensor(
                            xe[:], rp[:], -1.0, em[:], ALU.add, ALU.add)
                        # transpose -> Xt chunks
                        xt = hx.tile([P, 4, P], BF, tag="xt")
                        for c4 in range(4):
                            pt = ptP.tile([P, P], BF, tag="pt")
                            nc.tensor.transpose(pt[:], xe[:, c4 * P:(c4 + 1) * P],
                                                ident[:])
                            nc.scalar.copy(xt[:, c4, :], pt[:])
                        # next layer h_ext for this tile
                        h_tile(L + 1, t, xt)

            # ---- L2 edge loop ----
            if stage >= 4 and stage < 30:
                fire_ag(2, 1)
                pend = []
                for sp in range(min(KS, NS)):
                    pend.append(emit_glo(2, sp))
                for sp in range(NS):
                    g = pend.pop(0)
                    if sp + KS < NS:
                        pend.append(emit_glo(2, sp + KS))
                    gh = emit_ghi(2, sp)
                    for t in range(2 * sp, min(2 * sp + 2, T)):
                        ol = 0 if t == 2 * sp else BLt[2 * sp]
                        oh = 0 if t == 2 * sp else BHt[2 * sp]
                        gv = g[:, ol:ol + BLt[t], :]
                        ghv = gh[:, oh:oh + BHt[t], :]
                        xn = edge_tile(2, t, gv, ghv)
                        rows = min(P, PER - t * P)
                        nc.sync.dma_start(out_d[t * P:t * P + rows, :],
                                          xn[:rows, 0:NCLS])
            elif stage != 31:
                z0 = hx.tile([P, 512], F32, tag="xn")
                nc.vector.memset(z0[:], 0.0)
                for t in range(T):
                    rows = min(P, PER - t * P)
                    nc.sync.dma_start(out_d[t * P:t * P + rows, :],
                                      z0[:rows, 0:NCLS])

    nc.compile()
    nc.m = get_hw_module(nc.m)
    return nc


# ---------------------------------------------------------------------------
# Entry point
# ---------------------------------------------------------------------------

_CACHE = {}


def _get_nc(cfg, meta, stage=4):
    key = (tuple(sorted((k, str(v)) for k, v in cfg.items())),
           str(meta), stage)
    if key not in _CACHE:
        _CACHE[key] = build_gat_nc(cfg, meta, stage=stage)
    return _CACHE[key]


def run(inputs, trace=False, stage=4):
    cfg = real_cfg()
    in_maps, meta = host_prepare(inputs, cfg)
    nc = _get_nc(cfg, meta, stage=stage)
    res = bass_utils.run_bass_kernel_spmd(
        nc, in_maps, core_ids=list(range(cfg["R"])), trace=trace)
    out = np.concatenate([res.results[r]["out"] for r in range(cfg["R"])], axis=0)
    return out[: cfg["N"]], res


def kernel(**inputs) -> np.ndarray:
    out, _ = run(inputs, trace=False)
    return out.astype(np.float32)


# revision 46
# speedup vs baseline: 1.0323x; 1.0323x over previous
"""GAT (3-layer, PyG-style) Trainium2 Bass kernel, sharded across 8 NeuronCores.

Destination-node range partition (graph parallel), one SPMD program.
Per layer each core computes h_ext = X_own @ [W | W.a_s | W.a_d] for its
nodes (bf16 matmuls; the next layer's h_ext is fused into the edge loop so
the AllGather of the halo table overlaps edge aggregation). Tables: L0/L2
bf16; L1 rows are 768B mixed-precision (512 fp8 h | 16B bf16 att) - fp8 on
L0 was too lossy. A separate bf16 hloc strip keeps self-loop rows full
precision. Edge aggregation per pair of dst tiles: one lo + one hi
supergather (dma_gather, 0-padded idx; single_packet=False required above
~64 descs/engine), host-prebuilt fp8 one-hot S/St matrices, al_d per edge
via St matmul from a persistent SBUF al_d strip, softmax weights
max(exp(z), exp(0.2 z)) on ACT, weighted rows on DVE (per-head broadcast),
and the scatter runs as S matmuls accumulating numerator + denominator in
PSUM, with the self-loop as one extra identity-matrix block. AllGathers are
Pool-triggered (HW requirement, collectives are Pool-engine-only); each
layer's hi-region trigger fires at its edge loop's head, where its CC-stream
wait costs nothing because the lead gathers block on the lo AG anyway.

kernel(**inputs) takes the FULL inputs and returns the FULL [N, 16] output.
"""

import sys

sys.path.insert(0, "/opt/trn_rl_repo")

import numpy as np

import concourse.bass as bass
import concourse.mybir as mybir
import concourse.tile as tile
from concourse import bacc
from concourse import bass_utils
from concourse.bass_interp import get_hw_module
from concourse.masks import make_identity
from concourse import library_config

F32 = mybir.dt.float32
BF = mybir.dt.bfloat16
FP8 = mybir.dt.float8e4
I16 = mybir.dt.int16
import ml_dtypes
NPBF = ml_dtypes.bfloat16
NPF8 = ml_dtypes.float8_e4m3
P = 128
ALU = mybir.AluOpType
ACTF = mybir.ActivationFunctionType


def real_cfg():
    R = 8
    N = 50000
    PER = N // R                      # 6250 nodes per core
    T = (PER + P - 1) // P            # 49 dst tiles per core
    return dict(
        R=R, N=N, PER=PER, T=T, NPAD=T * P,
        F_IN=128, HID=64, HEADS=8, N_CLASSES=16,
        NEG=0.2,
        SPLIT_T=25,                   # lo group = tiles 0..24
        K_LEAD=6,                     # lo-gathers issued K tiles ahead
    )


# ---------------------------------------------------------------------------
# Host-side preprocessing
# ---------------------------------------------------------------------------

def _wrap16(flat):
    """int16 index list -> dma_gather idx layout [128, n/16]."""
    n = flat.shape[-1]
    w = flat.reshape(flat.shape[:-1] + (n // 16, 16))
    w = np.swapaxes(w, -1, -2)
    reps = (1,) * (flat.ndim - 1) + (8, 1)
    return np.ascontiguousarray(np.tile(w, reps), np.int16)


def host_prepare(inputs, cfg):
    R, N, PER, T, NPAD = cfg["R"], cfg["N"], cfg["PER"], cfg["T"], cfg["NPAD"]
    F_IN, HID, HEADS, NCLS = cfg["F_IN"], cfg["HID"], cfg["HEADS"], cfg["N_CLASSES"]
    HC = HID * HEADS                  # 512
    SPLIT_T = cfg["SPLIT_T"]
    LO = SPLIT_T * P                  # 3968
    HI = NPAD - LO                    # 2304

    x = np.asarray(inputs["x"], np.float32)
    ei = np.asarray(inputs["edge_index"])
    src = ei[0].astype(np.int64)
    dst = ei[1].astype(np.int64)      # self-loops handled analytically on device

    core = dst // PER
    dloc = (dst - core * PER).astype(np.int64)
    sloc = (src % PER).astype(np.int64)
    srank = (src // PER).astype(np.int64)
    is_lo = sloc < LO
    tile_of = dloc // P

    # gather row id within the lo / hi tables
    grow = np.where(is_lo, srank * LO + sloc, srank * HI + (sloc - LO))

    # per (core, tile, group) counts -> per-tile block counts (max over cores)
    cl = np.zeros((R, T), np.int64)
    ch = np.zeros((R, T), np.int64)
    np.add.at(cl, (core[is_lo], tile_of[is_lo]), 1)
    np.add.at(ch, (core[~is_lo], tile_of[~is_lo]), 1)
    BLt = np.maximum(1, np.ceil(cl.max(axis=0) / P).astype(np.int64))  # [T]
    BHt = np.maximum(1, np.ceil(ch.max(axis=0) / P).astype(np.int64))
    Bt = BLt + BHt
    BLM, BHM, BTM = int(BLt.max()), int(BHt.max()), int(Bt.max())

    # flat layout offsets (shared by all cores; shapes must be SPMD-static)
    off_lo = np.concatenate([[0], np.cumsum(BLt * 8)]).astype(np.int64)
    off_hi = np.concatenate([[0], np.cumsum(BHt * 8)]).astype(np.int64)
    off_S = np.concatenate([[0], np.cumsum(Bt * P)]).astype(np.int64)
    off_m8 = np.concatenate([[0], np.cumsum(Bt * 8)]).astype(np.int64)
    off_m1 = np.concatenate([[0], np.cumsum(Bt)]).astype(np.int64)

    # order edges by (core, tile, group), positions within each group
    order = np.lexsort((~is_lo * 1, tile_of, core))
    g_s = grow[order]
    d_s = dloc[order]
    core_s = core[order]
    tile_s = tile_of[order]
    lo_s = is_lo[order]
    grp = core_s * (2 * T) + tile_s * 2 + (~lo_s).astype(np.int64)
    grp_start = np.searchsorted(grp, np.arange(R * T * 2), side="left")
    pos = np.arange(len(grp)) - grp_start[grp]

    # NOTE: pad slots use index 0 (gather real row 0). Trailing -1 indices
    # would be truncated by the ucode, but that corrupts SWDGE ring
    # accounting when many gathers are in flight (HW hang).
    ilo = np.zeros((R, int((BLt * P).sum())), np.int16)
    ihi = np.zeros((R, int((BHt * P).sum())), np.int16)
    slot_lo = np.concatenate([[0], np.cumsum(BLt * P)]).astype(np.int64)
    slot_hi = np.concatenate([[0], np.cumsum(BHt * P)]).astype(np.int64)

    lo_m = lo_s
    hi_m = ~lo_s
    ilo[core_s[lo_m], slot_lo[tile_s[lo_m]] + pos[lo_m]] = g_s[lo_m].astype(np.int16)
    ihi[core_s[hi_m], slot_hi[tile_s[hi_m]] + pos[hi_m]] = g_s[hi_m].astype(np.int16)

    # S (scatter, [slot, 128 dst]) / St (transposed) / masks, flat by tile.
    # slot index within tile t: lo slots [0, BLt*P), hi slots [BLt*P, Bt*P)
    nS = int((Bt * P).sum())
    S_np = np.zeros((R, P, nS), NPF8)
    St_np = np.zeros((R, P, nS), NPF8)
    fpos = np.where(lo_m, pos, BLt[tile_s] * P + pos)  # slot within tile
    blk = fpos // P
    lane = fpos % P
    S_cols = off_S[tile_s] + blk * P + (d_s - tile_s * P)
    S_np[core_s, lane, S_cols] = 1.0
    St_np[core_s, (d_s - tile_s * P), off_S[tile_s] + blk * P + lane] = 1.0

    # wrap idx per tile -> [128, BLt*8] blocks, concat -> [128, sum]
    ilo_w = np.concatenate(
        [_wrap16(ilo[:, slot_lo[t]:slot_lo[t + 1]]) for t in range(T)], axis=-1)
    ihi_w = np.concatenate(
        [_wrap16(ihi[:, slot_hi[t]:slot_hi[t + 1]]) for t in range(T)], axis=-1)

    # weight assembly: W'[f, :] = [W | W.a_src | W.a_dst]
    def wext(W, a_s, a_d, ncols):
        Fin = W.shape[0]
        H, C = a_s.shape
        Wr = W.reshape(Fin, H, C)
        We = np.zeros((Fin, ncols), np.float32)
        We[:, : H * C] = W
        We[:, H * C: H * C + H] = np.einsum("fhc,hc->fh", Wr, a_s)
        We[:, H * C + H: H * C + 2 * H] = np.einsum("fhc,hc->fh", Wr, a_d)
        return We

    W0e = wext(np.asarray(inputs["W0"], np.float32),
               np.asarray(inputs["a_s0"], np.float32),
               np.asarray(inputs["a_d0"], np.float32), 528).astype(NPBF)
    W1e = wext(np.asarray(inputs["W1"], np.float32),
               np.asarray(inputs["a_s1"], np.float32),
               np.asarray(inputs["a_d1"], np.float32), 528)
    W2e = wext(np.asarray(inputs["W2"], np.float32),
               np.asarray(inputs["a_s2"], np.float32),
               np.asarray(inputs["a_d2"], np.float32), 18)
    W1e_r = W1e.reshape(4, P, 528).transpose(1, 0, 2).astype(NPBF).copy()
    W2p = np.zeros((512, 32), np.float32)
    W2p[:, :18] = W2e
    W2e_r = W2p.reshape(4, P, 32).transpose(1, 0, 2).astype(NPBF).copy()

    def bext(b, ncols):
        be = np.zeros((1, ncols), np.float32)
        be[0, : b.shape[0]] = b
        return np.ascontiguousarray(np.broadcast_to(be, (P, ncols)))

    b0e = bext(np.asarray(inputs["b0"], np.float32), 528)
    b1e = bext(np.asarray(inputs["b1"], np.float32), 528)
    b2e = bext(np.asarray(inputs["b2"], np.float32), 32)

    in_maps = []
    for r in range(R):
        xt = np.zeros((F_IN, NPAD), np.float32)
        xt[:, :PER] = x[r * PER: (r + 1) * PER].T
        in_maps.append({
            "xt0": xt.astype(NPBF),
            "w0e": W0e, "w1e": W1e_r, "w2e": W2e_r,
            "b0e": b0e, "b1e": b1e, "b2e": b2e,
            "ilo": ilo_w[r], "ihi": ihi_w[r],
            "S": S_np[r], "St": St_np[r],
        })
    meta = dict(BLt=BLt.tolist(), BHt=BHt.tolist(), Bt=Bt.tolist(),
                BLM=BLM, BHM=BHM, BTM=BTM,
                off_lo=off_lo.tolist(), off_hi=off_hi.tolist(),
                off_S=off_S.tolist(), off_m8=off_m8.tolist(),
                off_m1=off_m1.tolist())
    return in_maps, meta


# ---------------------------------------------------------------------------
# Device program
# ---------------------------------------------------------------------------

def build_gat_nc(cfg, meta, stage=4):
    R, PER, T, NPAD = cfg["R"], cfg["PER"], cfg["T"], cfg["NPAD"]
    F_IN, HID, HEADS, NCLS = cfg["F_IN"], cfg["HID"], cfg["HEADS"], cfg["N_CLASSES"]
    NEG = cfg["NEG"]
    HC = HID * HEADS
    SPLIT_T = cfg["SPLIT_T"]
    LO = SPLIT_T * P
    HI = NPAD - LO
    K = cfg["K_LEAD"]
    BLt, BHt, Bt = meta["BLt"], meta["BHt"], meta["Bt"]
    BLM, BHM, BTM = meta["BLM"], meta["BHM"], meta["BTM"]
    off_lo, off_hi = meta["off_lo"], meta["off_hi"]
    off_S, off_m8, off_m1 = meta["off_S"], meta["off_m8"], meta["off_m1"]

    nc = bacc.Bacc("TRN2", target_bir_lowering=False, debug=False,
                   num_devices=R)

    xt0_d = nc.dram_tensor("xt0", [F_IN, NPAD], BF, kind="ExternalInput")
    w0e_d = nc.dram_tensor("w0e", [P, 528], BF, kind="ExternalInput")
    w1e_d = nc.dram_tensor("w1e", [P, 4, 528], BF, kind="ExternalInput")
    w2e_d = nc.dram_tensor("w2e", [P, 4, 32], BF, kind="ExternalInput")
    b0e_d = nc.dram_tensor("b0e", [P, 528], F32, kind="ExternalInput")
    b1e_d = nc.dram_tensor("b1e", [P, 528], F32, kind="ExternalInput")
    b2e_d = nc.dram_tensor("b2e", [P, 32], F32, kind="ExternalInput")
    ilo_d = nc.dram_tensor("ilo", [P, off_lo[-1]], I16, kind="ExternalInput")
    ihi_d = nc.dram_tensor("ihi", [P, off_hi[-1]], I16, kind="ExternalInput")
    S_d = nc.dram_tensor("S", [P, off_S[-1]], FP8, kind="ExternalInput")
    St_d = nc.dram_tensor("St", [P, off_S[-1]], FP8, kind="ExternalInput")
    out_d = nc.dram_tensor("out", [PER, NCLS], F32, kind="ExternalOutput")

    rg = [list(range(R))]
    ROWW = [640, 768, 128]            # table row width per layer (in elems)
    TDT = [BF, FP8, BF]               # table dtype per layer
    NC_L = [HC, HC, NCLS]             # value cols per layer
    NH_L = [HEADS, HEADS, 1]

    with tile.TileContext(nc) as tc:
        with (
            tc.tile_pool(name="pers", bufs=1) as pers,
            tc.tile_pool(name="ld", bufs=3) as ld,        # S/St/mask/loc loads
            tc.tile_pool(name="ldi", bufs=7) as ldi,  # idx loads
            tc.tile_pool(name="glo", bufs=5) as gloP,
            tc.tile_pool(name="ghi", bufs=3) as ghiP,
            tc.tile_pool(name="wk", bufs=2) as wk,        # small working tiles
            tc.tile_pool(name="gw", bufs=2) as gwP,
            tc.tile_pool(name="hx", bufs=2) as hx,        # hsb / xe tiles
            tc.tile_pool(name="po", bufs=2, space="PSUM") as poP,
            tc.tile_pool(name="ps_sm", bufs=2, space="PSUM") as psS,
            tc.tile_pool(name="ph", bufs=2, space="PSUM") as phP,
            tc.tile_pool(name="phb", bufs=1, space="PSUM") as phbP,
            tc.tile_pool(name="pt", bufs=1, space="PSUM") as ptP,
            tc.tile_pool(name="dram", bufs=1, space="DRAM") as dram,
        ):
            nc.gpsimd.load_library(library_config.mlp)

            # ---- persistent tiles ----
            ident = pers.tile([P, P], BF)
            make_identity(nc, ident[:])

            w0_sb = pers.tile([P, 528], BF)
            w1_sb = pers.tile([P, 4, 528], BF)
            w2_sb = pers.tile([P, 4, 32], BF)
            b0_sb = pers.tile([P, 528], F32)
            b1_sb = pers.tile([P, 528], F32)
            b2_sb = pers.tile([P, 32], F32)
            nc.sync.dma_start(w0_sb[:], w0e_d[:, :])
            nc.sync.dma_start(w1_sb[:], w1e_d[:, :, :])
            nc.sync.dma_start(w2_sb[:], w2e_d[:, :, :])
            nc.sync.dma_start(b0_sb[:], b0e_d[:, :])
            nc.sync.dma_start(b1_sb[:], b1e_d[:, :])
            nc.sync.dma_start(b2_sb[:], b2e_d[:, :])

            Xt0 = pers.tile([P, NPAD], BF)
            nc.sync.dma_start(Xt0[:], xt0_d[:, :])

            # al_d strips: [128, T*nH] per layer
            ald = [pers.tile([P, T * 8], BF, name=f"ald{L}") for L in range(3)]
            for L in range(3):
                nc.vector.memset(ald[L][:], 0.0)

            # ---- internal DRAM ----
            hA = [dram.tile([LO, ROWW[L]], TDT[L], name=f"hA{L}") for L in range(3)]
            hB = [dram.tile([HI, ROWW[L]], TDT[L], name=f"hB{L}") for L in range(3)]
            tA = [dram.tile([R * LO, ROWW[L]], TDT[L], addr_space="Shared",
                            name=f"tA{L}") for L in range(3)]
            tB = [dram.tile([R * HI, ROWW[L]], TDT[L], addr_space="Shared",
                            name=f"tB{L}") for L in range(3)]
            hloc = {1: dram.tile([NPAD, 528], BF, name="hloc1")}


            def fire_ag(L, region):
                ins_ap, outs_ap = ((hA[L][:, :], tA[L][:, :]) if region == 0
                                   else (hB[L][:, :], tB[L][:, :]))
                nc.gpsimd.collective_compute(
                    "AllGather", ALU.bypass, replica_groups=rg,
                    ins=[ins_ap], outs=[outs_ap])

            def h_tile(L, nt, src_sb):
                """h_ext for layer L, dst tile nt, from feature-major src_sb
                ([P, kc, P] kc chunks of lhsT). Writes table row block + ald."""
                roww = ROWW[L]
                ncol = NC_L[L]
                nh = NH_L[L]
                tot = ncol + 2 * nh
                W_sb = [w0_sb, w1_sb, w2_sb][L]
                b_sb = [b0_sb, b1_sb, b2_sb][L]
                KC = 1 if L == 0 else 4
                pha = phP.tile([P, 512], F32, tag="pha")
                phb = phbP.tile([P, 64], F32, tag="phb")
                n1 = min(512, tot)
                for kc in range(KC):
                    lhs = (Xt0[:, nt * P:(nt + 1) * P] if L == 0
                           else src_sb[:, kc, :])
                    rhs = (w0_sb[:, 0:n1] if L == 0
                           else W_sb[:, kc, 0:n1])
                    nc.tensor.matmul(pha[:, 0:n1], lhsT=lhs, rhs=rhs,
                                     start=(kc == 0), stop=(kc == KC - 1))
                if tot > 512:
                    for kc in range(KC):
                        lhs = (Xt0[:, nt * P:(nt + 1) * P] if L == 0
                               else src_sb[:, kc, :])
                        rhs = (w0_sb[:, 512:tot] if L == 0
                               else W_sb[:, kc, 512:tot])
                        nc.tensor.matmul(phb[:, 0:tot - 512], lhsT=lhs, rhs=rhs,
                                         start=(kc == 0), stop=(kc == KC - 1))
                hsb = hx.tile([P, 528], BF, tag="hsb")
                if tot > 512:
                    nc.vector.tensor_tensor(hsb[:, 0:512], pha[:, 0:512],
                                            b_sb[:, 0:512], ALU.add)
                    nc.vector.tensor_tensor(hsb[:, 512:tot], phb[:, 0:tot - 512],
                                            b_sb[:, 512:tot], ALU.add)
                else:
                    nc.vector.tensor_tensor(hsb[:, 0:tot], pha[:, 0:tot],
                                            b_sb[:, 0:tot], ALU.add)
                # al_d strip
                nc.scalar.copy(ald[L][:, nt * 8:nt * 8 + nh],
                               hsb[:, ncol + nh:tot])
                # table row block
                if TDT[L] == FP8:
                    f8 = hx.tile([P, 512], FP8, tag="hf8")
                    nc.scalar.activation(f8[:], hsb[:, 0:512], ACTF.Copy)
                    att8 = hsb[:, 512:528].bitcast(FP8)
                    dstT = hA[L] if nt < SPLIT_T else hB[L]
                    r0 = nt * P if nt < SPLIT_T else nt * P - LO
                    nc.sync.dma_start(dstT[r0:r0 + P, 0:512], f8[:])
                    nc.sync.dma_start(dstT[r0:r0 + P, 512:544], att8)
                    nc.sync.dma_start(hloc[L][nt * P:(nt + 1) * P, :],
                                      hsb[:, 0:528])
                elif nt < SPLIT_T:
                    nc.sync.dma_start(hA[L][nt * P:(nt + 1) * P, 0:tot],
                                      hsb[:, 0:tot])
                else:
                    r0 = nt * P - LO
                    nc.sync.dma_start(hB[L][r0:r0 + P, 0:tot], hsb[:, 0:tot])
                if nt == SPLIT_T - 1:
                    fire_ag(L, 0)

            # =========== L0 h phase (standalone) ===========
            for nt in range(T):
                h_tile(0, nt, None)

            # =========== edge phases (fused with next layer h) ===========
            def emit_glo(L, s):
                # supergather: one gather covering tiles 2s and 2s+1
                roww = ROWW[L]
                t0 = 2 * s
                t1 = min(2 * s + 2, T)
                bl = sum(BLt[t0:t1])
                it = ldi.tile([P, 2 * BLM * 8], I16, tag="ilo")
                nc.sync.dma_start(it[:, 0:bl * 8], ilo_d[:, off_lo[t0]:off_lo[t1]])
                g = gloP.tile([P, 2 * BLM, roww], TDT[L], tag="glo")
                nc.gpsimd.dma_gather(
                    g[:, 0:bl, :], tA[L][:, :], it[:, 0:bl * 8],
                    num_idxs=bl * P, num_idxs_reg=bl * P, elem_size=roww,
                    single_packet=False)
                return g

            def emit_ghi(L, s):
                roww = ROWW[L]
                t0 = 2 * s
                t1 = min(2 * s + 2, T)
                bh = sum(BHt[t0:t1])
                it = ldi.tile([P, 2 * BHM * 8], I16, tag="ihi")
                nc.sync.dma_start(it[:, 0:bh * 8], ihi_d[:, off_hi[t0]:off_hi[t1]])
                gh = ghiP.tile([P, 2 * BHM, roww], TDT[L], tag="ghi")
                nc.gpsimd.dma_gather(
                    gh[:, 0:bh, :], tB[L][:, :], it[:, 0:bh * 8],
                    num_idxs=bh * P, num_idxs_reg=bh * P, elem_size=roww,
                    single_packet=False)
                return gh

            def edge_tile(L, t, g, gh):
                roww = ROWW[L]
                ncol = NC_L[L]
                nh = NH_L[L]
                tot = ncol + 2 * nh
                bl, bh, bt = BLt[t], BHt[t], Bt[t]
                alow = ncol            # al_s col offset in table rows

                S_sb = ld.tile([P, BTM * P], FP8, tag="S")
                St_sb = ld.tile([P, BTM * P], FP8, tag="St")
                nc.sync.dma_start(S_sb[:, 0:bt * P], S_d[:, off_S[t]:off_S[t + 1]])
                nc.sync.dma_start(St_sb[:, 0:bt * P], St_d[:, off_S[t]:off_S[t + 1]])
                loc = ld.tile([P, 528], BF, tag="loc")
                if TDT[L] == FP8:
                    nc.sync.dma_start(loc[:, 0:tot],
                                      hloc[L][t * P:(t + 1) * P, 0:tot])
                elif t < SPLIT_T:
                    nc.sync.dma_start(loc[:, 0:tot], hA[L][t * P:(t + 1) * P, 0:tot])
                else:
                    r0 = t * P - LO
                    nc.sync.dma_start(loc[:, 0:tot], hB[L][r0:r0 + P, 0:tot])

                # ---- al_d per edge: psmall[:, b*nh:(b+1)*nh] = St_b.T @ ald ----
                psm = psS.tile([P, (BTM + 1) * 8], F32, tag="psm")
                for b in range(bt):
                    nc.tensor.matmul(
                        psm[:, b * nh:(b + 1) * nh],
                        lhsT=St_sb[:, b * P:(b + 1) * P],
                        rhs=ald[L][:, t * 8:t * 8 + nh],
                        start=True, stop=True)

                # ---- logits, weights ----
                z = wk.tile([P, BTM * 8], F32, tag="z")
                if TDT[L] == FP8:
                    galsv_lo = g[:, 0:bl, :].bitcast(BF)[:, :, 256:256 + nh]
                    galsv_hi = gh[:, 0:bh, :].bitcast(BF)[:, :, 256:256 + nh]
                else:
                    galsv_lo = g[:, 0:bl, alow:alow + nh]
                    galsv_hi = gh[:, 0:bh, alow:alow + nh]
                nc.vector.tensor_tensor(
                    z[:, 0:bl * nh].rearrange("p (b h) -> p b h", b=bl),
                    galsv_lo, psm[:, 0:bl * nh].rearrange("p (b h) -> p b h", b=bl),
                    ALU.add)
                nc.vector.tensor_tensor(
                    z[:, bl * nh:bt * nh].rearrange("p (b h) -> p b h", b=bh),
                    galsv_hi,
                    psm[:, bl * nh:bt * nh].rearrange("p (b h) -> p b h", b=bh),
                    ALU.add)
                e1 = wk.tile([P, BTM * 8], F32, tag="e1")
                e2 = wk.tile([P, BTM * 8], F32, tag="e2")
                nc.scalar.activation(e1[:, 0:bt * nh], z[:, 0:bt * nh], ACTF.Exp)
                nc.scalar.activation(e2[:, 0:bt * nh], z[:, 0:bt * nh], ACTF.Exp,
                                     scale=NEG)
                gw = gwP.tile([P, BTM + 1, 520], BF, tag="gw")
                # weights -> gw[:, b, 512:512+nh] (pad slots killed by S zeros)
                nc.vector.tensor_tensor(
                    gw[:, 0:bt, 512:512 + nh],
                    e1[:, 0:bt * nh].rearrange("p (b h) -> p b h", b=bt),
                    e2[:, 0:bt * nh].rearrange("p (b h) -> p b h", b=bt),
                    ALU.max)

                # ---- self-loop weight ----
                sl = wk.tile([P, 3 * 8], F32, tag="sl")
                nc.vector.tensor_tensor(sl[:, 0:nh], loc[:, alow:alow + nh],
                                        ald[L][:, t * 8:t * 8 + nh], ALU.add)
                nc.scalar.activation(sl[:, 8:8 + nh], sl[:, 0:nh], ACTF.Exp)
                nc.scalar.activation(sl[:, 16:16 + nh], sl[:, 0:nh], ACTF.Exp,
                                     scale=NEG)
                nc.vector.tensor_tensor(gw[:, bt, 512:512 + nh],
                                        sl[:, 8:8 + nh], sl[:, 16:16 + nh],
                                        ALU.max)

                # ---- weighted rows ----
                nc.vector.tensor_tensor(
                    gw[:, 0:bl, 0:ncol].rearrange("p b (h c) -> p b h c", h=nh),
                    g[:, 0:bl, 0:ncol].rearrange("p b (h c) -> p b h c", h=nh),
                    gw[:, 0:bl, 512:512 + nh]
                        .unsqueeze(3).to_broadcast([P, bl, nh, ncol // nh]),
                    ALU.mult)
                nc.vector.tensor_tensor(
                    gw[:, bl:bt, 0:ncol].rearrange("p b (h c) -> p b h c", h=nh),
                    gh[:, 0:bh, 0:ncol].rearrange("p b (h c) -> p b h c", h=nh),
                    gw[:, bl:bt, 512:512 + nh]
                        .unsqueeze(3).to_broadcast([P, bh, nh, ncol // nh]),
                    ALU.mult)
                nc.vector.tensor_tensor(
                    gw[:, bt, 0:ncol].rearrange("p (h c) -> p h c", h=nh),
                    loc[:, 0:ncol].rearrange("p (h c) -> p h c", h=nh),
                    gw[:, bt, 512:512 + nh]
                        .unsqueeze(2).to_broadcast([P, nh, ncol // nh]),
                    ALU.mult)

                # ---- scatter matmuls: numerator + denominator ----
                po = poP.tile([P, 512], F32, tag="po")
                for b in range(bt + 1):
                    lhs = (S_sb[:, b * P:(b + 1) * P] if b < bt else ident[:])
                    nc.tensor.matmul(po[:, 0:ncol], lhsT=lhs,
                                     rhs=gw[:, b, 0:ncol],
                                     start=(b == 0), stop=(b == bt))
                    nc.tensor.matmul(psm[:, BTM * 8:BTM * 8 + nh], lhsT=lhs,
                                     rhs=gw[:, b, 512:512 + nh],
                                     start=(b == 0), stop=(b == bt))

                rden = wk.tile([P, 8], F32, tag="rden")
                nc.vector.reciprocal(rden[:, 0:nh], psm[:, BTM * 8:BTM * 8 + nh])
                xn = hx.tile([P, 512], F32, tag="xn")
                nc.vector.tensor_tensor(
                    xn[:, 0:ncol].rearrange("p (h c) -> p h c", h=nh),
                    po[:, 0:ncol].rearrange("p (h c) -> p h c", h=nh),
                    rden[:, 0:nh].unsqueeze(2).to_broadcast([P, nh, ncol // nh]),
                    ALU.mult)
                return xn

            if stage == 31:
                # debug: dump tA[0] rows 0:128 (fp8) as f32 into out
                dbg = pers.tile([P, 768], FP8, name="dbg8")
                dbgf = pers.tile([P, 768], F32, name="dbgf")
                nc.sync.dma_start(dbg[:], tA[0][0:P, :])
                nc.scalar.activation(dbgf[:], dbg[:], ACTF.Copy)
                for j in range(47):
                    nc.sync.dma_start(out_d[j * P:(j + 1) * P, :],
                                      dbgf[:, j * 16:(j + 1) * 16])
                dbga = pers.tile([P, 16], F32, name="dbga")
                nc.scalar.activation(dbga[:], dbg[:].bitcast(BF)[:, 256:272],
                                     ACTF.Copy)
                nc.sync.dma_start(out_d[47 * P:48 * P, :], dbga[:])

            # ---- L0 / L1 edge loops (fused with next h) ----
            NS = (T + 1) // 2
            KS = 3
            for L in ((0, 1) if (stage >= 3 and stage < 30) else ((0,) if 20 <= stage < 30 else ())):
                # hi-region AG for THIS layer's table: the lead gathers below
                # block on the lo-region AG anyway, so firing first costs
                # nothing and starts the hi AG as soon as the CC stream frees
                fire_ag(L, 1)
                pend = []
                for sp in range(min(KS, NS)):
                    pend.append(emit_glo(L, sp))
                for sp in range(NS):
                    g = pend.pop(0)
                    if sp + KS < NS:
                        pend.append(emit_glo(L, sp + KS))
                    gh = emit_ghi(L, sp)
                    for t in range(2 * sp, min(2 * sp + 2, T)):
                        ol = 0 if t == 2 * sp else BLt[2 * sp]
                        oh = 0 if t == 2 * sp else BHt[2 * sp]
                        gv = g[:, ol:ol + BLt[t], :]
                        ghv = gh[:, oh:oh + BHt[t], :]
                        xn = edge_tile(L, t, gv, ghv)
                        # ELU -> xe (bf16)
                        m = wk.tile([P, 512], F32, tag="elu_m")
                        nc.scalar.activation(m[:], xn[:], ACTF.Relu, scale=-1.0)
                        em = wk.tile([P, 512], F32, tag="elu_e")
                        nc.scalar.activation(em[:], m[:], ACTF.Exp, scale=-1.0)
                        rp = wk.tile([P, 512], F32, tag="elu_r")
                        nc.scalar.activation(rp[:], xn[:], ACTF.Relu)
                        xe = hx.tile([P, 512], BF, tag="xe")
                        nc.vector.scalar_tensor_tensor(
                            xe[:], rp[:], -1.0, em[:], ALU.add, ALU.add)
                        # transpose -> Xt chunks
                        xt = hx.tile([P, 4, P], BF, tag="xt")
                        for c4 in range(4):
                            pt = ptP.tile([P, P], BF, tag="pt")
                            nc.tensor.transpose(pt[:], xe[:, c4 * P:(c4 + 1) * P],
                                                ident[:])
                            nc.scalar.copy(xt[:, c4, :], pt[:])
                        # next layer h_ext for this tile
                        h_tile(L + 1, t, xt)

            # ---- L2 edge loop ----
            if stage >= 4 and stage < 30:
                fire_ag(2, 1)
                pend = []
                for sp in range(min(KS, NS)):
                    pend.append(emit_glo(2, sp))
                for sp in range(NS):
                    g = pend.pop(0)
                    if sp + KS < NS:
                        pend.append(emit_glo(2, sp + KS))
                    gh = emit_ghi(2, sp)
                    for t in range(2 * sp, min(2 * sp + 2, T)):
                        ol = 0 if t == 2 * sp else BLt[2 * sp]
                        oh = 0 if t == 2 * sp else BHt[2 * sp]
                        gv = g[:, ol:ol + BLt[t], :]
                        ghv = gh[:, oh:oh + BHt[t], :]
                        xn = edge_tile(2, t, gv, ghv)
                        rows = min(P, PER - t * P)
                        nc.sync.dma_start(out_d[t * P:t * P + rows, :],
                                          xn[:rows, 0:NCLS])
            elif stage != 31:
                z0 = hx.tile([P, 512], F32, tag="xn")
                nc.vector.memset(z0[:], 0.0)
                for t in range(T):
                    rows = min(P, PER - t * P)
                    nc.sync.dma_start(out_d[t * P:t * P + rows, :],
                                      z0[:rows, 0:NCLS])

    nc.compile()
    nc.m = get_hw_module(nc.m)
    return nc


# ---------------------------------------------------------------------------
# Entry point
# ---------------------------------------------------------------------------

_CACHE = {}


def _get_nc(cfg, meta, stage=4):
    key = (tuple(sorted((k, str(v)) for k, v in cfg.items())),
           str(meta), stage)
    if key not in _CACHE:
        _CACHE[key] = build_gat_nc(cfg, meta, stage=stage)
    return _CACHE[key]


def run(inputs, trace=False, stage=4):
    cfg = real_cfg()
    in_maps, meta = host_prepare(inputs, cfg)
    nc = _get_nc(cfg, meta, stage=stage)
    res = bass_utils.run_bass_kernel_spmd(
        nc, in_maps, core_ids=list(range(cfg["R"])), trace=trace)
    out = np.concatenate([res.results[r]["out"] for r in range(cfg["R"])], axis=0)
    return out[: cfg["N"]], res


def kernel(**inputs) -> np.ndarray:
    out, _ = run(inputs, trace=False)
    return out.astype(np.float32)
